# revision 19
# baseline (speedup 1.0000x reference)
"""Trainium2 Bass kernel for nn_CrossAttention1D_78640851190158.

Math: k/v in the MHA come from a single cond token broadcast to all T
key positions, so the softmax over identical scores is exactly uniform
and the attention output equals v2 broadcast over T. The whole module
collapses to

    out[b, c, t] = x[b, c, t] + y[b, c]
    y[b] = W_eff @ cond[b] + b_eff

where W_eff = proj_w @ out_w @ wv2 @ Wv  (wv2 = in_proj_w[2C:]) and
b_eff folds all the biases through the same chain. The LayerNorm / q
path contributes nothing to the output for ANY input values.

Sharding: pure data parallelism over batch B=8 across the 8 cores.
Each core computes y for its batch with two DVE ops (multiply by a
broadcast cond + grouped reduce, bias folded in as an extra column)
and streams its 2 MB x slice through SBUF with a broadcast add
(memory-bound: ~4.7 MB HBM traffic/core).
"""

import numpy as np

B, C, T, COND = 8, 512, 1024, 256
N_CORES = 8
# x[b] (C*T floats) viewed as [128, 4096]: partition p holds channels
# 4p..4p+3 as four contiguous 1024-wide quarters.
P, F = 128, C * T // 128
QW = T  # quarter width == chunk size
NQ = 4
KB = COND + 1  # cond extended with 1.0 to fold the bias in

_cache = {}


def build_kernel():
    import concourse.mybir as mybir
    from concourse import bacc
    from concourse.tile import TileContext

    f32 = mybir.dt.float32
    # Bacc (not plain Bass): its compile() runs generate_event_semaphores,
    # which splits multi-sem waits to satisfy TRN2's 1-wait-per-instruction
    # constraint. Plain Bass BIR fails walrus codegen with
    # "Too many sync wait commands".
    nc = bacc.Bacc()

    x_d = nc.dram_tensor("x", [P, F], f32, kind="ExternalInput")
    # packed per-core constants, loaded as one DMA per quarter:
    #   [p, q*KB + j] = W_eff[4p+q, j] for j < COND;  j = COND -> b_eff[4p+q]
    #   [p, NQ*KB + k] = cond[b][k] for k < COND; k = COND -> 1.0
    #   (cond block replicated on every partition)
    WCOLS = NQ * KB + KB
    w_d = nc.dram_tensor("wconst", [P, WCOLS], f32, kind="ExternalInput")
    out_d = nc.dram_tensor("out", [P, F], f32, kind="ExternalOutput")

    with TileContext(nc) as tc:
        with (
            tc.tile_pool(name="w", bufs=1) as wpool,
            tc.tile_pool(name="xp", bufs=NQ) as xpool,
        ):
            w_sb = wpool.tile([P, WCOLS], f32)
            tmp_sb = wpool.tile([P, NQ * KB], f32)
            y_sb = wpool.tile([P, NQ], f32)

            # cond + per-quarter weight chunks first (0.64 MB total) so
            # the whole y chain unblocks before the x stream lands; w_q
            # split lets y_q start at w_q's completion, not the last one's.
            nc.sync.dma_start(out=w_sb[:, NQ * KB :], in_=w_d[:, NQ * KB :])
            for h in range(NQ):
                nc.sync.dma_start(
                    out=w_sb[:, h * KB : (h + 1) * KB],
                    in_=w_d[:, h * KB : (h + 1) * KB],
                )
            xts = []
            for h in range(NQ):
                xt = xpool.tile([P, QW], f32, tag="xt")
                nc.sync.dma_start(out=xt[:], in_=x_d[:, h * QW : (h + 1) * QW])
                xts.append(xt)

            # y_sb[p, q] = sum_j W_eff[4p+q, j]*cond[j] + b_eff[4p+q]
            # (tensor_tensor_reduce and 3D grouped reduces crash the HW
            # runtime here; plain 2D mult + reduce per quarter is safe)
            for q in range(NQ):
                nc.vector.tensor_tensor(
                    out=tmp_sb[:, q * KB : (q + 1) * KB],
                    in0=w_sb[:, q * KB : (q + 1) * KB],
                    in1=w_sb[:, NQ * KB :],
                    op=mybir.AluOpType.mult,
                )
                nc.vector.tensor_reduce(
                    out=y_sb[:, q : q + 1],
                    in_=tmp_sb[:, q * KB : (q + 1) * KB],
                    axis=mybir.AxisListType.X,
                    op=mybir.AluOpType.add,
                )

            # stream x through SBUF one quarter (512 KB) at a time.
            # out-DMAs ride the ACT HWDGE ring (nc.scalar), so their
            # descriptor generation runs parallel to the SP ring's loads
            # and stores never queue behind later loads in ring FIFO.
            for h in range(NQ):
                xo = xpool.tile([P, QW], f32, tag="xo")
                nc.vector.tensor_scalar_add(
                    out=xo[:],
                    in0=xts[h][:],
                    scalar1=y_sb[:, h : h + 1],
                )
                nc.scalar.dma_start(out=out_d[:, h * QW : (h + 1) * QW], in_=xo[:])

    nc.compile()
    return nc


def fold_weights(Wv, bv, in_proj_w, in_proj_b, out_w, out_b, proj_w, proj_b):
    """Fold the v-path weight chain into one [C, COND] map (float64)."""
    wv2 = np.asarray(in_proj_w, np.float64)[2 * C :]
    bv2 = np.asarray(in_proj_b, np.float64)[2 * C :]
    Wv = np.asarray(Wv, np.float64)
    bv = np.asarray(bv, np.float64)
    out_w = np.asarray(out_w, np.float64)
    out_b = np.asarray(out_b, np.float64)
    proj_w = np.asarray(proj_w, np.float64)
    proj_b = np.asarray(proj_b, np.float64)

    po = proj_w @ out_w
    W_eff = po @ wv2 @ Wv
    b_eff = proj_b + proj_w @ out_b + po @ bv2 + po @ wv2 @ bv
    return W_eff.astype(np.float32), b_eff.astype(np.float32)


def prepare_in_maps(inputs):
    x = np.ascontiguousarray(np.asarray(inputs["x"], np.float32))
    cond = np.ascontiguousarray(np.asarray(inputs["cond"], np.float32))
    W_eff, b_eff = fold_weights(
        inputs["Wv"], inputs["bv"], inputs["in_proj_w"], inputs["in_proj_b"],
        inputs["out_w"], inputs["out_b"], inputs["proj_w"], inputs["proj_b"],
    )
    # weights+bias block: [p, q*KB + j] = W_eff[4p+q, j], col j=COND = b_eff
    wblk = np.concatenate(
        [W_eff.reshape(P, NQ, COND), b_eff.reshape(P, NQ, 1)], axis=2
    ).reshape(P, NQ * KB)
    in_maps = []
    for b in range(B):
        cond_ext = np.concatenate([cond[b], [np.float32(1.0)]]).astype(np.float32)
        cond_blk = np.broadcast_to(cond_ext, (P, KB))
        wconst = np.ascontiguousarray(
            np.concatenate([wblk, cond_blk], axis=1, dtype=np.float32)
        )
        in_maps.append({"x": x[b].reshape(P, F), "wconst": wconst})
    return in_maps


def kernel(**inputs):
    from concourse.bass_utils import run_bass_kernel_spmd

    if "nc" not in _cache:
        _cache["nc"] = build_kernel()
    nc = _cache["nc"]
    in_maps = prepare_in_maps(inputs)
    res = run_bass_kernel_spmd(nc, in_maps, list(range(N_CORES)))
    out = np.stack([r["out"].reshape(C, T) for r in res.results])
    return out.astype(np.float32)


# revision 20
# speedup vs baseline: 1.0293x; 1.0293x over previous
"""Trainium2 Bass kernel for nn_CrossAttention1D_78640851190158.

Math: k/v in the MHA come from a single cond token broadcast to all T
key positions, so the softmax over identical scores is exactly uniform
and the attention output equals v2 broadcast over T. The whole module
collapses to

    out[b, c, t] = x[b, c, t] + y[b, c]
    y[b] = W_eff @ cond[b] + b_eff

where W_eff = proj_w @ out_w @ wv2 @ Wv  (wv2 = in_proj_w[2C:]) and
b_eff folds all the biases through the same chain. The LayerNorm / q
path contributes nothing to the output for ANY input values.

Sharding: pure data parallelism over batch B=8 across the 8 cores.
Each core computes y for its batch with two DVE ops (multiply by a
broadcast cond + grouped reduce, bias folded in as an extra column)
and streams its 2 MB x slice through SBUF with a broadcast add
(memory-bound: ~4.7 MB HBM traffic/core).
"""

import numpy as np

B, C, T, COND = 8, 512, 1024, 256
N_CORES = 8
# x[b] (C*T floats) viewed as [128, 4096]: partition p holds channels
# 4p..4p+3 as four contiguous 1024-wide quarters.
P, F = 128, C * T // 128
QW = T  # quarter width == chunk size
NQ = 4
KB = COND + 1  # cond extended with 1.0 to fold the bias in

_cache = {}


def build_kernel():
    import concourse.mybir as mybir
    from concourse import bacc
    from concourse.tile import TileContext

    f32 = mybir.dt.float32
    # Bacc (not plain Bass): its compile() runs generate_event_semaphores,
    # which splits multi-sem waits to satisfy TRN2's 1-wait-per-instruction
    # constraint. Plain Bass BIR fails walrus codegen with
    # "Too many sync wait commands".
    nc = bacc.Bacc()

    x_d = nc.dram_tensor("x", [P, F], f32, kind="ExternalInput")
    # packed per-core constants, loaded as one DMA per quarter:
    #   [p, q*KB + j] = W_eff[4p+q, j] for j < COND;  j = COND -> b_eff[4p+q]
    #   [p, NQ*KB + k] = cond[b][k] for k < COND; k = COND -> 1.0
    #   (cond block replicated on every partition)
    WCOLS = NQ * KB + KB
    w_d = nc.dram_tensor("wconst", [P, WCOLS], f32, kind="ExternalInput")
    out_d = nc.dram_tensor("out", [P, F], f32, kind="ExternalOutput")

    with TileContext(nc) as tc:
        with (
            tc.tile_pool(name="w", bufs=1) as wpool,
            tc.tile_pool(name="xp", bufs=NQ) as xpool,
        ):
            w_sb = wpool.tile([P, WCOLS], f32)
            tmp_sb = wpool.tile([P, NQ * KB], f32)
            y_sb = wpool.tile([P, NQ], f32)

            # single w DMA first (0.66 MB), then the x stream — extra DMA
            # instructions cost ~0.6 us serialized descriptor-gen each on
            # the SP sequencer, so fewer/bigger transfers win.
            nc.sync.dma_start(out=w_sb[:], in_=w_d[:])
            xts = []
            for h in range(NQ):
                xt = xpool.tile([P, QW], f32, tag="xt")
                nc.sync.dma_start(out=xt[:], in_=x_d[:, h * QW : (h + 1) * QW])
                xts.append(xt)

            # y_sb[p, q] = sum_j W_eff[4p+q, j]*cond[j] + b_eff[4p+q]
            # (tensor_tensor_reduce and 3D grouped reduces crash the HW
            # runtime here; plain 2D mult + reduce per quarter is safe)
            for q in range(NQ):
                nc.vector.tensor_tensor(
                    out=tmp_sb[:, q * KB : (q + 1) * KB],
                    in0=w_sb[:, q * KB : (q + 1) * KB],
                    in1=w_sb[:, NQ * KB :],
                    op=mybir.AluOpType.mult,
                )
                nc.vector.tensor_reduce(
                    out=y_sb[:, q : q + 1],
                    in_=tmp_sb[:, q * KB : (q + 1) * KB],
                    axis=mybir.AxisListType.X,
                    op=mybir.AluOpType.add,
                )

            # stream x through SBUF one quarter (512 KB) at a time.
            # out-DMAs ride the ACT HWDGE ring (nc.scalar), so their
            # descriptor generation runs parallel to the SP ring's loads
            # and stores never queue behind later loads in ring FIFO.
            for h in range(NQ):
                xo = xpool.tile([P, QW], f32, tag="xo")
                nc.vector.tensor_scalar_add(
                    out=xo[:],
                    in0=xts[h][:],
                    scalar1=y_sb[:, h : h + 1],
                )
                nc.scalar.dma_start(out=out_d[:, h * QW : (h + 1) * QW], in_=xo[:])

    nc.compile()
    return nc


def fold_weights(Wv, bv, in_proj_w, in_proj_b, out_w, out_b, proj_w, proj_b):
    """Fold the v-path weight chain into one [C, COND] map (float64)."""
    wv2 = np.asarray(in_proj_w, np.float64)[2 * C :]
    bv2 = np.asarray(in_proj_b, np.float64)[2 * C :]
    Wv = np.asarray(Wv, np.float64)
    bv = np.asarray(bv, np.float64)
    out_w = np.asarray(out_w, np.float64)
    out_b = np.asarray(out_b, np.float64)
    proj_w = np.asarray(proj_w, np.float64)
    proj_b = np.asarray(proj_b, np.float64)

    po = proj_w @ out_w
    W_eff = po @ wv2 @ Wv
    b_eff = proj_b + proj_w @ out_b + po @ bv2 + po @ wv2 @ bv
    return W_eff.astype(np.float32), b_eff.astype(np.float32)


def prepare_in_maps(inputs):
    x = np.ascontiguousarray(np.asarray(inputs["x"], np.float32))
    cond = np.ascontiguousarray(np.asarray(inputs["cond"], np.float32))
    W_eff, b_eff = fold_weights(
        inputs["Wv"], inputs["bv"], inputs["in_proj_w"], inputs["in_proj_b"],
        inputs["out_w"], inputs["out_b"], inputs["proj_w"], inputs["proj_b"],
    )
    # weights+bias block: [p, q*KB + j] = W_eff[4p+q, j], col j=COND = b_eff
    wblk = np.concatenate(
        [W_eff.reshape(P, NQ, COND), b_eff.reshape(P, NQ, 1)], axis=2
    ).reshape(P, NQ * KB)
    in_maps = []
    for b in range(B):
        cond_ext = np.concatenate([cond[b], [np.float32(1.0)]]).astype(np.float32)
        cond_blk = np.broadcast_to(cond_ext, (P, KB))
        wconst = np.ascontiguousarray(
            np.concatenate([wblk, cond_blk], axis=1, dtype=np.float32)
        )
        in_maps.append({"x": x[b].reshape(P, F), "wconst": wconst})
    return in_maps


def kernel(**inputs):
    from concourse.bass_utils import run_bass_kernel_spmd

    if "nc" not in _cache:
        _cache["nc"] = build_kernel()
    nc = _cache["nc"]
    in_maps = prepare_in_maps(inputs)
    res = run_bass_kernel_spmd(nc, in_maps, list(range(N_CORES)))
    out = np.stack([r["out"].reshape(C, T) for r in res.results])
    return out.astype(np.float32)
